# revision 13
# baseline (speedup 1.0000x reference)
"""Trainium2 Bass kernel for the vq_codebook problem (prototype learning with
masked sinkhorn), data-parallel over the token dim N on 8 NeuronCores.

Self-contained: hardcodes shapes (N=200704, D=512, K=4 classes, M=8 protos).

Math restructure (validated against the jax reference in numpy):
  - feats are streamed d-major (host-transposed) through the TensorEngine in
    fp32r; LN + l2-normalize are folded into per-token output-domain
    corrections:  masks = w*(x@P'^T) - (w*mu)*psum' + s*const,
    with per-token stats (Sx, Sx*g^2, Sx*g*b, Sx^2, Sx^2*g^2) produced by the
    same matmuls via extra stationary columns.
  - sinkhorn col-normalizations need global column sums -> 3 tiny AllReduces;
    the initial total/selcount fold into the first one. Row norms are local.
  - q = onehot(argmax) realized as equality-with-max (exact ties are
    measure-zero and masked rows are killed by m_k).
  - f = m_q^T c_q restructured as (A'^T x - A'^T mu) with A' = q*m_k*r*s in
    bf16; aux stationary columns [mu, 1, invr, u] give the correction term,
    sum(a), and an exact-zero-preserving count proxy. One 66KB AllReduce,
    then the replicated EMA + l2norm on every core.
"""

from contextlib import ExitStack

import numpy as np

import concourse.bass as bass
import concourse.bacc as bacc
import concourse.mybir as mybir
import concourse.tile as tile
from concourse.bass_utils import run_bass_kernel_spmd
from concourse.masks import make_identity

F32 = mybir.dt.float32
F32R = mybir.dt.float32r
BF16 = mybir.dt.bfloat16
I32 = mybir.dt.int32
AX = mybir.AxisListType.X
OP = mybir.AluOpType
AF = mybir.ActivationFunctionType

N_FULL = 200704
D = 512
K = 4
M = 8
KM = K * M                      # 32
NC_CORES = 8
NLOC = N_FULL // NC_CORES       # 25088
GAMMA = 0.999
EPS_SINK = 0.05
RR = 38                         # psum rows: 32 masks | Sx | Sxg2 | Sxgb | Sxx | Sxxg2 | pad
CH = D // 128                   # 4 d-chunks


def build(nloc=NLOC, num_cores=NC_CORES, has_beta=False):
    TILES = nloc // 128
    GROUPS = nloc // 512
    assert nloc % 512 == 0

    nc = bacc.Bacc("TRN2", target_bir_lowering=False, debug=False,
                   num_devices=num_cores)

    xt_d = nc.dram_tensor("xt", [D, nloc], F32R, kind="ExternalInput").ap()
    xbf_d = nc.dram_tensor("xbf", [nloc, D], BF16, kind="ExternalInput").ap()
    gt_d = nc.dram_tensor("gt", [128, TILES], I32, kind="ExternalInput").ap()
    g_d = nc.dram_tensor("g", [D], F32, kind="ExternalInput").ap()
    b_d = nc.dram_tensor("b", [D], F32, kind="ExternalInput").ap()
    mg_d = nc.dram_tensor("mg", [K], F32, kind="ExternalInput").ap()
    mb_d = nc.dram_tensor("mb", [K], F32, kind="ExternalInput").ap()
    pr_d = nc.dram_tensor("protos", [KM, D], F32, kind="ExternalInput").ap()
    blka_d = nc.dram_tensor("blka", [KM, K], F32, kind="ExternalInput").ap()
    blkb_d = nc.dram_tensor("blkb", [K, KM], F32, kind="ExternalInput").ap()
    oseg_d = nc.dram_tensor("out_seg", [128, TILES * K], F32,
                            kind="ExternalOutput").ap()
    newp_d = nc.dram_tensor("new_protos", [KM, D], F32,
                            kind="ExternalOutput").ap()

    rg = [list(range(num_cores))]

    with tile.TileContext(nc) as tc:
        _body(nc, tc, xt_d, xbf_d, gt_d, g_d, b_d, mg_d, mb_d, pr_d,
              blka_d, blkb_d, oseg_d, newp_d, nloc, TILES, GROUPS, rg, has_beta)
    nc.compile()
    return nc


def _body(nc, tc, xt_d, xbf_d, gt_d, g_d, b_d, mg_d, mb_d, pr_d,
          blka_d, blkb_d, oseg_d, newp_d, nloc, TILES, GROUPS, rg, has_beta):
    f32r = lambda ap: ap.bitcast(F32R)
    vec, sca, ten = nc.vector, nc.scalar, nc.tensor
    TT, TS, STT = vec.tensor_tensor, vec.tensor_scalar, vec.scalar_tensor_tensor

    ctx = ExitStack()
    cp = ctx.enter_context(tc.tile_pool(name="const", bufs=1))
    slab = ctx.enter_context(tc.tile_pool(name="slab", bufs=1))
    psS = ctx.enter_context(tc.tile_pool(name="psS", bufs=2, space="PSUM"))
    dramp = ctx.enter_context(tc.tile_pool(name="dram", bufs=1, space="DRAM"))

    # ---------------- stage 0: constants ----------------
    ident = cp.tile([128, 128], F32, tag="ident")
    make_identity(nc, ident[:])
    ones_1x128 = cp.tile([1, 128], F32, tag="ones1")
    vec.memset(ones_1x128[:], 1.0)
    ones_128x1 = cp.tile([128, 1], F32, tag="ones128")
    vec.memset(ones_128x1[:], 1.0)
    ones_r = cp.tile([128, 1], F32R, tag="ones_r")
    vec.tensor_copy(ones_r[:], ones_128x1[:])

    def bcast_row(dst_sb, src_row, n):
        """[1, n] SBUF row -> [128, n] SBUF (all partitions)."""
        ps = psS.tile([128, 512], F32, tag="ps_s", name="ps_bc")
        ten.matmul(ps[:, 0:n], ones_1x128[:], src_row, start=True, stop=True)
        vec.tensor_copy(dst_sb, ps[:, 0:n])

    # gamma/beta in [32, D] broadcast and [128, CH] chunk-column layouts
    g_row = cp.tile([1, D], F32, tag="g_row")
    b_row = cp.tile([1, D], F32, tag="b_row")
    nc.sync.dma_start(g_row[:], g_d.rearrange("(a d) -> a d", a=1))
    nc.sync.dma_start(b_row[:], b_d.rearrange("(a d) -> a d", a=1))
    g_pb = cp.tile([KM, D], F32, tag="g_pb")
    b_pb = cp.tile([KM, D], F32, tag="b_pb")
    ps_gb = psS.tile([128, 512], F32, tag="ps_s")
    ten.matmul(ps_gb[0:KM, :], ones_1x128[:, 0:KM], g_row[:], start=True, stop=True)
    vec.tensor_copy(g_pb[:], ps_gb[0:KM, :])
    ten.matmul(ps_gb[0:KM, :], ones_1x128[:, 0:KM], b_row[:], start=True, stop=True)
    vec.tensor_copy(b_pb[:], ps_gb[0:KM, :])

    gcols = cp.tile([128, CH], F32, tag="gcols")
    bcols = cp.tile([128, CH], F32, tag="bcols")
    nc.sync.dma_start(gcols[:], g_d.rearrange("(c p) -> p c", p=128))
    nc.sync.dma_start(bcols[:], b_d.rearrange("(c p) -> p c", p=128))
    g2cols = cp.tile([128, CH], F32, tag="g2cols")
    gbcols = cp.tile([128, CH], F32, tag="gbcols")
    TT(g2cols[:], gcols[:], gcols[:], OP.mult)
    TT(gbcols[:], gcols[:], bcols[:], OP.mult)

    # prototypes: l2 normalize rows -> Pn; P' = g * Pn
    pr_sb = cp.tile([KM, D], F32, tag="pr_sb")
    nc.sync.dma_start(pr_sb[:], pr_d)
    scratchKD = cp.tile([KM, D], F32, tag="scrKD")
    s1 = cp.tile([KM, 1], F32, tag="s1")
    s2 = cp.tile([KM, 1], F32, tag="s2")
    sca.activation(scratchKD[:], pr_sb[:], AF.Square, accum_out=s1[:])
    sca.activation(s2[:], s1[:], AF.Sqrt)
    TS(s1[:], s2[:], 1e-12, None, OP.max)
    vec.reciprocal(s2[:], s1[:])
    protos_n = cp.tile([KM, D], F32, tag="protos_n")
    TS(protos_n[:], pr_sb[:], s2[:], None, OP.mult)
    Pp = cp.tile([KM, D], F32, tag="Pp")
    TT(Pp[:], protos_n[:], g_pb[:], OP.mult)

    # lhsT tiles for the streaming matmuls
    lhsx = []
    lhsxx = []
    ps_t1 = psS.tile([128, 512], F32, tag="ps_s")
    lst = cp.tile([128, RR], F32, tag="lst")
    for c in range(CH):
        lx = cp.tile([128, RR], F32R, tag=f"lhsx{c}")
        ten.transpose(ps_t1[:, 0:KM], Pp[:, c * 128:(c + 1) * 128],
                      ident[0:KM, 0:KM])
        vec.tensor_copy(lst[:, 0:KM], ps_t1[:, 0:KM])
        vec.memset(lst[:, 32:33], 1.0)
        vec.tensor_copy(lst[:, 33:34], g2cols[:, c:c + 1])
        vec.tensor_copy(lst[:, 34:35], gbcols[:, c:c + 1])
        vec.memset(lst[:, 35:38], 0.0)
        vec.tensor_copy(lx[:], lst[:])
        lhsx.append(lx)
        lxx = cp.tile([128, RR], F32R, tag=f"lhsxx{c}")
        vec.memset(lst[:], 0.0)
        vec.memset(lst[:, 35:36], 1.0)
        vec.tensor_copy(lst[:, 36:37], g2cols[:, c:c + 1])
        vec.tensor_copy(lxx[:], lst[:])
        lhsxx.append(lxx)

    # constrow: per-column sums of lhsx over d -> [1, 38]:
    #   cols 0..31 psum'_m | 512 | Sg2 | Sgb | 0 | 0 | col 37 = Sb2
    ps_cr = psS.tile([1, 512], F32, tag="ps_s")
    for c in range(CH):
        ten.matmul(ps_cr[0:1, 0:RR], ones_r[:], lhsx[c][:],
                   start=(c == 0), stop=(c == CH - 1))
    constrow = cp.tile([1, 38], F32, tag="constrow")
    vec.tensor_copy(constrow[:, 0:RR], ps_cr[0:1, 0:RR])
    sb2 = cp.tile([1, 1], F32, tag="sb2")
    scr1D = cp.tile([1, D], F32, tag="scr1D")
    sca.activation(scr1D[:], b_row[:], AF.Square, accum_out=sb2[:])
    vec.tensor_copy(constrow[:, 37:38], sb2[:])
    const_b = cp.tile([128, 38], F32, tag="const_b")
    bcast_row(const_b[:], constrow[:], 38)
    negP2b = cp.tile([128, KM], F32, tag="negP2b")
    TS(negP2b[:], const_b[:, 0:KM], -1.0, None, OP.mult)

    # constm (only needed when beta != 0): constm_row[m] = b . Pn_m
    constm_b = None
    if has_beta:
        ps_cm = psS.tile([1, 512], F32, tag="ps_s")
        for c in range(CH):
            ten.transpose(ps_t1[:, 0:KM], protos_n[:, c * 128:(c + 1) * 128],
                          ident[0:KM, 0:KM])
            pnt = cp.tile([128, KM], F32, tag="pnt")
            vec.tensor_copy(pnt[:], ps_t1[:, 0:KM])
            ten.matmul(ps_cm[0:1, 0:KM], bcols[:, c:c + 1], pnt[:],
                       start=(c == 0), stop=(c == CH - 1))
        cm_row = cp.tile([1, KM], F32, tag="cm_row")
        vec.tensor_copy(cm_row[:], ps_cm[0:1, 0:KM])
        constm_b = cp.tile([128, KM], F32, tag="constm_b")
        bcast_row(constm_b[:], cm_row[:], KM)

    # mask gamma/beta broadcast [128, 4]
    mg_row = cp.tile([1, K], F32, tag="mg_row")
    mb_row = cp.tile([1, K], F32, tag="mb_row")
    nc.sync.dma_start(mg_row[:], mg_d.rearrange("(a k) -> a k", a=1))
    nc.sync.dma_start(mb_row[:], mb_d.rearrange("(a k) -> a k", a=1))
    mg_b = cp.tile([128, K], F32, tag="mg_b")
    mb_b = cp.tile([128, K], F32, tag="mb_b")
    bcast_row(mg_b[:], mg_row[:], K)
    bcast_row(mb_b[:], mb_row[:], K)

    # block-diag helpers for per-class partition sums/broadcasts (host consts)
    blkA = cp.tile([KM, K], F32, tag="blkA")      # [32,4]: 1 if p//8==k
    blkB = cp.tile([K, KM], F32, tag="blkB")      # [4,32]: 1 if m//8==k
    nc.sync.dma_start(blkA[:], blka_d)
    nc.sync.dma_start(blkB[:], blkb_d)

    # ---------------- stage 1: stream feats, matmuls, transposes ----------------
    raw_slab = slab.tile([128, TILES * RR], F32, tag="raw")
    raw3 = raw_slab[:].rearrange("p (t r) -> p t r", r=RR)

    st1ctx = ExitStack()
    st1 = st1ctx.enter_context(tc.tile_pool(name="st1", bufs=8))
    st1xx = st1ctx.enter_context(tc.tile_pool(name="st1xx", bufs=4))
    st1m = st1ctx.enter_context(tc.tile_pool(name="st1m", bufs=3))
    psA = st1ctx.enter_context(tc.tile_pool(name="psA", bufs=2, space="PSUM"))
    psT = st1ctx.enter_context(tc.tile_pool(name="psT", bufs=2, space="PSUM"))

    for gr in range(GROUPS):
        xts = []
        for c in range(CH):
            xt_t = st1.tile([128, 512], F32R, tag="xt_t")
            nc.sync.dma_start(xt_t[:], xt_d[c * 128:(c + 1) * 128,
                                            gr * 512:(gr + 1) * 512])
            xts.append(xt_t)
        mps = psA.tile([RR, 512], F32, tag="mps")
        for c in range(CH):
            xx_t = st1xx.tile([128, 512], F32R, tag="xx_t")
            sca.activation(xx_t[:], xts[c][:], AF.Square)
            ten.matmul(mps[:], lhsx[c][:], xts[c][:],
                       start=(c == 0), stop=False)
            ten.matmul(mps[:], lhsxx[c][:], xx_t[:],
                       start=False, stop=(c == CH - 1))
        mT_sb = st1m.tile([RR, 512], F32, tag="mT_sb")
        sca.copy(mT_sb[:], mps[:])
        tp = psT.tile([128, 4 * RR], F32, tag="tp")
        for j in range(4):
            ten.transpose(tp[:, j * RR:(j + 1) * RR],
                          mT_sb[:, j * 128:(j + 1) * 128], ident[0:RR, 0:RR])
        vec.tensor_copy(raw_slab[:, gr * 4 * RR:(gr + 1) * 4 * RR], tp[:])

    # ---------------- stage 1.5: per-token stats ----------------
    def tokarr(tag):
        return slab.tile([128, TILES], F32, tag=tag, name=tag)

    Sx, Sxg2, Sxgb, Sxx, Sxxg2 = (raw3[:, :, 32], raw3[:, :, 33],
                                  raw3[:, :, 34], raw3[:, :, 35], raw3[:, :, 36])
    mu = tokarr("mu")
    TS(mu[:], Sx, 1.0 / D, None, OP.mult)
    mu2 = tokarr("mu2")
    TT(mu2[:], mu[:], mu[:], OP.mult)
    var = tokarr("var")
    TS(var[:], Sxx, 1.0 / D, None, OP.mult)
    TT(var[:], var[:], mu2[:], OP.subtract)
    sd = tokarr("sd")                      # sqrt(var+eps) == 1/r == invr
    TS(var[:], var[:], 1e-5, None, OP.add)
    sca.activation(sd[:], var[:], AF.Sqrt)
    r_ = tokarr("r_")
    vec.reciprocal(r_[:], sd[:])
    # zn2 = r^2*(Sxxg2 - 2 mu Sxg2 + mu^2 Sg2) + 2 r (Sxgb - mu Sgb) + Sb2
    t1 = tokarr("t1")
    t2 = tokarr("t2")
    TT(t1[:], mu[:], Sxg2, OP.mult)                                   # mu*Sxg2
    STT(t1[:], t1[:], -2.0, Sxxg2, OP.mult, OP.add)                   # Sxxg2-2muSxg2
    STT(t1[:], mu2[:], const_b[:, 33:34], t1[:], OP.mult, OP.add)     # +mu^2*Sg2
    TT(t2[:], r_[:], r_[:], OP.mult)
    TT(t1[:], t1[:], t2[:], OP.mult)                                  # zn2 part a
    zn2 = t1
    if has_beta:
        STT(t2[:], mu[:], const_b[:, 34:35], Sxgb, OP.mult, OP.subtract)  # muSgb-Sxgb
        TT(t2[:], t2[:], r_[:], OP.mult)
        STT(zn2[:], t2[:], -2.0, zn2[:], OP.mult, OP.add)
        TS(zn2[:], zn2[:], const_b[:, 37:38], None, OP.add)
    sz = tokarr("sz")
    sca.activation(sz[:], zn2[:], AF.Sqrt)
    TS(sz[:], sz[:], 1e-12, None, OP.max)
    s_ = tokarr("s_")
    vec.reciprocal(s_[:], sz[:])
    w_ = tokarr("w_")                       # w = r*s  (== w2, the a' scale)
    TT(w_[:], r_[:], s_[:], OP.mult)
    wmu = tokarr("wmu")
    TT(wmu[:], w_[:], mu[:], OP.mult)
    uaux = tokarr("uaux")                   # 1/w = sd*sz
    TT(uaux[:], sd[:], sz[:], OP.mult)

    # ---------------- stage 1.6: masks correction ----------------
    masks_slab = slab.tile([128, TILES * KM], F32, tag="masks")
    masks3 = masks_slab[:].rearrange("p (t m) -> p t m", m=KM)
    tmp_slab = slab.tile([128, TILES * KM], F32, tag="tmpslab")
    tmp3 = tmp_slab[:].rearrange("p (t m) -> p t m", m=KM)
    wb = w_[:].unsqueeze(2).broadcast_to([128, TILES, KM])
    TT(masks3, raw3[:, :, 0:KM], wb, OP.mult)
    wmub = wmu[:].unsqueeze(2).broadcast_to([128, TILES, KM])
    negP2bb = negP2b[:].unsqueeze(1).broadcast_to([128, TILES, KM])
    TT(tmp3, wmub, negP2bb, OP.mult)
    TT(masks3, masks3, tmp3, OP.add)
    if has_beta:
        sb_ = s_[:].unsqueeze(2).broadcast_to([128, TILES, KM])
        cmb = constm_b[:].unsqueeze(1).broadcast_to([128, TILES, KM])
        TT(tmp3, sb_, cmb, OP.mult)
        TT(masks3, masks3, tmp3, OP.add)

    # ---------------- stage 1.7: out_seg + pred + sel ----------------
    masks4 = masks_slab[:].rearrange("p (t k m) -> p t k m", k=K, m=M)
    mx = slab.tile([128, TILES * K], F32, tag="mx")
    mx3 = mx[:].rearrange("p (t k) -> p t k", k=K)
    vec.tensor_reduce(mx3, masks4, AX, OP.max)
    mu4 = tokarr("mu4")
    vec.tensor_reduce(mu4[:], mx3, AX, OP.add)
    TS(mu4[:], mu4[:], 1.0 / K, None, OP.mult)
    d4 = slab.tile([128, TILES * K], F32, tag="d4")
    d43 = d4[:].rearrange("p (t k) -> p t k", k=K)
    mu4b = mu4[:].unsqueeze(2).broadcast_to([128, TILES, K])
    TT(d43, mx3, mu4b, OP.subtract)
    sq4 = slab.tile([128, TILES * K], F32, tag="sq4")
    sca.activation(sq4[:], d4[:], AF.Square)
    v4 = tokarr("v4")
    vec.tensor_reduce(v4[:], sq4[:].rearrange("p (t k) -> p t k", k=K), AX, OP.add)
    TS(v4[:], v4[:], 1.0 / K, 1e-5, OP.mult, OP.add)
    sd4 = tokarr("sd4")
    sca.activation(sd4[:], v4[:], AF.Sqrt)
    rs4 = tokarr("rs4")
    vec.reciprocal(rs4[:], sd4[:])
    oseg = slab.tile([128, TILES * K], F32, tag="oseg")
    oseg3 = oseg[:].rearrange("p (t k) -> p t k", k=K)
    rs4b = rs4[:].unsqueeze(2).broadcast_to([128, TILES, K])
    TT(oseg3, d43, rs4b, OP.mult)
    mgb = mg_b[:].unsqueeze(1).broadcast_to([128, TILES, K])
    mbb = mb_b[:].unsqueeze(1).broadcast_to([128, TILES, K])
    TT(oseg3, oseg3, mgb, OP.mult)
    TT(oseg3, oseg3, mbb, OP.add)
    nc.sync.dma_start(oseg_d, oseg[:])

    m4x = tokarr("m4x")
    vec.tensor_reduce(m4x[:], oseg3, AX, OP.max)
    eqp = slab.tile([128, TILES * K], F32, tag="eqp")
    eqp3 = eqp[:].rearrange("p (t k) -> p t k", k=K)
    m4xb = m4x[:].unsqueeze(2).broadcast_to([128, TILES, K])
    TT(eqp3, oseg3, m4xb, OP.is_equal)

    gt_sb = cp.tile([128, TILES], I32, tag="gt_sb")
    nc.sync.dma_start(gt_sb[:], gt_d)
    gtf = tokarr("gtf")
    vec.tensor_copy(gtf[:], gt_sb[:])
    sel4 = slab.tile([128, TILES * K], F32, tag="sel4")
    sel43 = sel4[:].rearrange("p (t k) -> p t k", k=K)
    for k in range(K):
        TS(sel43[:, :, k], gtf[:], float(k), None, OP.is_equal)
    mk = slab.tile([128, TILES * K], F32, tag="mk")
    TT(mk[:], eqp[:], sel4[:], OP.mult)

    selcnt = cp.tile([128, K], F32, tag="selcnt")
    vec.tensor_reduce(selcnt[:], sel4[:].rearrange("p (t k) -> p k t", k=K),
                      AX, OP.add)

    # ---------------- stage 2: sinkhorn ----------------
    L3 = masks3                     # L overwrites masks in place
    L4 = masks4
    Lcol = masks_slab[:].rearrange("p (t m) -> p m t", m=KM)
    sca.activation(masks_slab[:], masks_slab[:], AF.Exp, scale=1.0 / EPS_SINK)
    selb8 = sel43.unsqueeze(3).broadcast_to([128, TILES, K, M])
    TT(L4, L4, selb8, OP.mult)

    colpart = cp.tile([128, KM + K], F32, tag="colpart")
    row = slab.tile([128, TILES * K], F32, tag="sq4")
    row3 = row[:].rearrange("p (t k) -> p t k", k=K)
    rowfac = slab.tile([128, TILES * K], F32, tag="d4")
    rowfac3 = rowfac[:].rearrange("p (t k) -> p t k", k=K)

    arA_in = dramp.tile([1, KM + K], F32, tag="arA_in")
    arA_out = dramp.tile([1, KM + K], F32, tag="arA_out")
    arB_in = dramp.tile([1, KM], F32, tag="arB_in")
    arB_out = dramp.tile([1, KM], F32, tag="arB_out")
    arC_in = dramp.tile([1, KM], F32, tag="arC_in")
    arC_out = dramp.tile([1, KM], F32, tag="arC_out")

    invB_b = cp.tile([128, K], F32, tag="invB_b")
    colfac_b = cp.tile([128, KM], F32, tag="colfac_b")

    # iteration 1: colsum0 + selcount in one AllReduce
    vec.tensor_reduce(colpart[:, 0:KM], Lcol, AX, OP.add)
    vec.tensor_copy(colpart[:, KM:KM + K], selcnt[:])
    ps_c = psS.tile([1, 512], F32, tag="ps_s")
    ten.matmul(ps_c[0:1, 0:KM + K], ones_128x1[:], colpart[:],
               start=True, stop=True)
    arA_sb = cp.tile([1, KM + K], F32, tag="arA_sb")
    vec.tensor_copy(arA_sb[:], ps_c[0:1, 0:KM + K])
    nc.sync.dma_start(arA_in[:], arA_sb[:])
    nc.gpsimd.collective_compute("AllReduce", OP.add, replica_groups=rg,
                                 ins=[arA_in[:].opt()], outs=[arA_out[:].opt()])
    nc.sync.dma_start(arA_sb[:], arA_out[:])

    # small algebra on partition 0: colfac1 = 1/(C*M*T), C = max(cs/T,1e-30)
    cs_k = arA_sb[:, 0:KM].rearrange("a (k m) -> a k m", k=K)
    Tk = cp.tile([1, K], F32, tag="Tk")
    vec.tensor_reduce(Tk[:], cs_k, AX, OP.add)
    TS(Tk[:], Tk[:], 1e-30, None, OP.max)
    rTk = cp.tile([1, K], F32, tag="rTk")
    vec.reciprocal(rTk[:], Tk[:])
    Cv = cp.tile([1, KM], F32, tag="Cv")
    rTb = rTk[:].unsqueeze(2).broadcast_to([1, K, M])
    TT(Cv[:].rearrange("a (k m) -> a k m", k=K), cs_k, rTb, OP.mult)
    TS(Cv[:], Cv[:], 1e-30, None, OP.max)
    rCv = cp.tile([1, KM], F32, tag="rCv")
    vec.reciprocal(rCv[:], Cv[:])
    colfac = cp.tile([1, KM], F32, tag="colfac")
    TT(colfac[:].rearrange("a (k m) -> a k m", k=K), rCv[:].rearrange(
        "a (k m) -> a k m", k=K), rTb, OP.mult)
    TS(colfac[:], colfac[:], 1.0 / M, None, OP.mult)
    # B = max(selc,1); invB
    Bk = cp.tile([1, K], F32, tag="Bk")
    TS(Bk[:], arA_sb[:, KM:KM + K], 1.0, None, OP.max)
    invB = cp.tile([1, K], F32, tag="invB")
    vec.reciprocal(invB[:], Bk[:])
    bcast_row(invB_b[:], invB[:], K)

    for it in range(3):
        if it > 0:
            # colfac = 1/(max(colsum,1e-30)*M) from the fresh AllReduce
            ar_sb = cp.tile([1, KM], F32, tag="ar_sb")
            nc.sync.dma_start(ar_sb[:], (arB_out if it == 1 else arC_out)[:])
            TS(ar_sb[:], ar_sb[:], 1e-30, None, OP.max)
            vec.reciprocal(colfac[:], ar_sb[:])
            TS(colfac[:], colfac[:], 1.0 / M, None, OP.mult)
        bcast_row(colfac_b[:], colfac[:], KM)
        cfb = colfac_b[:].unsqueeze(1).broadcast_to([128, TILES, KM])
        TT(L3, L3, cfb, OP.mult)
        if it == 2:
            break
        vec.tensor_reduce(row3, L4, AX, OP.add)
        TS(row[:], row[:], 1e-30, None, OP.max)
        vec.reciprocal(rowfac[:], row[:])
        TT(rowfac[:], rowfac[:], sel4[:], OP.mult)
        invBb = invB_b[:].unsqueeze(1).broadcast_to([128, TILES, K])
        TT(rowfac3, rowfac3, invBb, OP.mult)
        rfb = rowfac3.unsqueeze(3).broadcast_to([128, TILES, K, M])
        TT(L4, L4, rfb, OP.mult)
        # next colsum partial + AllReduce
        vec.tensor_reduce(colpart[:, 0:KM], Lcol, AX, OP.add)
        ps_c2 = psS.tile([1, 512], F32, tag="ps_s")
        ten.matmul(ps_c2[0:1, 0:KM], ones_128x1[:],
                   colpart[:, 0:KM], start=True, stop=True)
        ar_next_sb = cp.tile([1, KM], F32, tag="arN_sb")
        vec.tensor_copy(ar_next_sb[:], ps_c2[0:1, 0:KM])
        ar_in, ar_out = (arB_in, arB_out) if it == 0 else (arC_in, arC_out)
        nc.sync.dma_start(ar_in[:], ar_next_sb[:])
        nc.gpsimd.collective_compute("AllReduce", OP.add, replica_groups=rg,
                                     ins=[ar_in[:].opt()], outs=[ar_out[:].opt()])

    # ---------------- stage 3: q, A', aux, f matmul ----------------
    maxL = slab.tile([128, TILES * K], F32, tag="mx")
    maxL3 = maxL[:].rearrange("p (t k) -> p t k", k=K)
    vec.tensor_reduce(maxL3, L4, AX, OP.max)
    mLb = maxL3.unsqueeze(3).broadcast_to([128, TILES, K, M])
    TT(L4, L4, mLb, OP.is_equal)          # L becomes eqL in place
    v2 = slab.tile([128, TILES * K], F32, tag="eqp")
    wb2 = w_[:].unsqueeze(2).broadcast_to([128, TILES, K])
    TT(v2[:].rearrange("p (t k) -> p t k", k=K), mk[:].rearrange(
        "p (t k) -> p t k", k=K), wb2, OP.mult)
    v2b = v2[:].rearrange("p (t k) -> p t k", k=K).unsqueeze(3).broadcast_to(
        [128, TILES, K, M])
    TT(L4, L4, v2b, OP.mult)              # L becomes A' (fp32) in place
    A_bf = slab.tile([128, TILES * KM], BF16, tag="A_bf")
    vec.tensor_copy(A_bf[:], masks_slab[:])

    aux_slab = slab.tile([128, TILES * 4], BF16, tag="aux")
    aux3 = aux_slab[:].rearrange("p (t c) -> p t c", c=4)
    vec.tensor_copy(aux3[:, :, 0], mu[:])
    vec.memset(aux3[:, :, 1], 1.0)
    vec.tensor_copy(aux3[:, :, 2], sd[:])
    vec.tensor_copy(aux3[:, :, 3], uaux[:])

    # close stage-1 pools before opening the f-phase input pool
    st1ctx.close()

    A3 = A_bf[:].rearrange("p (t m) -> p t m", m=KM)
    psF = ctx.enter_context(tc.tile_pool(name="psF", bufs=1, space="PSUM"))
    xbfp = ctx.enter_context(tc.tile_pool(name="xbfp", bufs=16))
    fps = psF.tile([KM, D], F32, tag="fps")
    auxps = psF.tile([KM, 4], F32, tag="auxps")
    for t in range(TILES):
        xb_t = xbfp.tile([128, D], BF16, tag="xb_t")
        nc.sync.dma_start(xb_t[:], xbf_d[t * 128:(t + 1) * 128, :])
        ten.matmul(fps[:], A3[:, t, :], xb_t[:], start=(t == 0),
                   stop=(t == TILES - 1))
        ten.matmul(auxps[:], A3[:, t, :], aux3[:, t, :], start=(t == 0),
                   stop=(t == TILES - 1))

    fsb = cp.tile([KM, D + 4], F32, tag="fsb")
    vec.tensor_copy(fsb[:, 0:D], fps[:])
    vec.tensor_copy(fsb[:, D:D + 4], auxps[:])
    arD_in = dramp.tile([KM, D + 4], F32, tag="arD_in")
    arD_out = dramp.tile([KM, D + 4], F32, tag="arD_out")
    nc.sync.dma_start(arD_in[:], fsb[:])
    nc.gpsimd.collective_compute("AllReduce", OP.add, replica_groups=rg,
                                 ins=[arD_in[:].opt()], outs=[arD_out[:].opt()])
    nc.sync.dma_start(fsb[:], arD_out[:])

    # ---------------- stage 4: replicated EMA update ----------------
    fK = cp.tile([KM, D], F32, tag="fK")
    # f = (fsb - t1) * g + sa * b ; t1 = col D, sa = col D+2, nprox = col D+3
    TS(fK[:], fsb[:, 0:D], fsb[:, D:D + 1], None, OP.subtract)
    TT(fK[:], fK[:], g_pb[:], OP.mult)
    if has_beta:
        STT(fK[:], b_pb[:], fsb[:, D + 2:D + 3], fK[:], OP.mult, OP.add)
    # l2 normalize f
    sca.activation(scratchKD[:], fK[:], AF.Square, accum_out=s1[:])
    sca.activation(s2[:], s1[:], AF.Sqrt)
    TS(s1[:], s2[:], 1e-12, None, OP.max)
    vec.reciprocal(s2[:], s1[:])
    TS(fK[:], fK[:], s2[:], None, OP.mult)
    # upd = gamma*protos_n + (1-gamma)*fn
    upd = cp.tile([KM, D], F32, tag="upd")
    TS(upd[:], fK[:], 1.0 - GAMMA, None, OP.mult)
    STT(upd[:], protos_n[:], GAMMA, upd[:], OP.mult, OP.add)
    # valid = (nprox != 0) & (selc > 0) & (sum_m nprox > 0)
    nprox = cp.tile([KM, 1], F32, tag="nprox")
    vec.tensor_copy(nprox[:], fsb[:, D + 3:D + 4])
    ps_v = psS.tile([KM, 512], F32, tag="ps_s")
    ten.matmul(ps_v[0:K, 0:1], blkA[:], nprox[:], start=True, stop=True)
    nk4 = cp.tile([K, 1], F32, tag="nk4")
    vec.tensor_copy(nk4[:], ps_v[0:K, 0:1])
    ten.matmul(ps_v[0:KM, 0:1], blkB[:], nk4[:], start=True, stop=True)
    nkb = cp.tile([KM, 1], F32, tag="nkb")
    vec.tensor_copy(nkb[:], ps_v[0:KM, 0:1])
    # selcount per class broadcast to [KM, 1]
    sel4p = cp.tile([K, 1], F32, tag="sel4p")
    nc.sync.dma_start(sel4p[:], arA_out[:, KM:KM + K].rearrange("a k -> k a"))
    ten.matmul(ps_v[0:KM, 0:1], blkB[:], sel4p[:], start=True, stop=True)
    selb = cp.tile([KM, 1], F32, tag="selb")
    vec.tensor_copy(selb[:], ps_v[0:KM, 0:1])
    valid = cp.tile([KM, 1], F32, tag="valid")
    TS(valid[:], nprox[:], 0.0, None, OP.not_equal)
    vb = cp.tile([KM, 1], F32, tag="vb")
    TS(vb[:], nkb[:], 0.0, None, OP.is_gt)
    TT(valid[:], valid[:], vb[:], OP.mult)
    TS(vb[:], selb[:], 0.0, None, OP.is_gt)
    TT(valid[:], valid[:], vb[:], OP.mult)
    # newp = protos_n + valid*(upd - protos_n), then l2 normalize
    newp = cp.tile([KM, D], F32, tag="newp")
    TT(newp[:], upd[:], protos_n[:], OP.subtract)
    TS(newp[:], newp[:], valid[:], None, OP.mult)
    TT(newp[:], newp[:], protos_n[:], OP.add)
    sca.activation(scratchKD[:], newp[:], AF.Square, accum_out=s1[:])
    sca.activation(s2[:], s1[:], AF.Sqrt)
    TS(s1[:], s2[:], 1e-12, None, OP.max)
    vec.reciprocal(s2[:], s1[:])
    TS(newp[:], newp[:], s2[:], None, OP.mult)
    nc.sync.dma_start(newp_d, newp[:])

    ctx.close()


def _to_f32r(a):
    """Round fp32 to the fp32r grid (e8m11; low 12 mantissa bits zero, RNE)."""
    u = np.ascontiguousarray(a, np.float32).view(np.uint32)
    low = u & np.uint32(0xFFF)
    hi = u >> np.uint32(12)
    carry = (low > 0x800) | ((low == 0x800) & ((hi & np.uint32(1)) == 1))
    return ((hi + carry.astype(np.uint32)) << np.uint32(12)).view(np.float32)


_NC_CACHE = {}
TRACE = False
TRACE_DIR = None
LAST_RES = None


def _get_nc():
    key = "full"
    if key not in _NC_CACHE:
        _NC_CACHE[key] = build()
    return _NC_CACHE[key]


def kernel(**inputs):
    import ml_dtypes
    feats = np.ascontiguousarray(np.asarray(inputs["feats"], dtype=np.float32))
    gt = np.asarray(inputs["gt_seg"]).astype(np.int32)
    g = np.ascontiguousarray(np.asarray(inputs["ln_gamma"], dtype=np.float32))
    b = np.ascontiguousarray(np.asarray(inputs["ln_beta"], dtype=np.float32))
    mg = np.ascontiguousarray(np.asarray(inputs["mask_gamma"], dtype=np.float32))
    mb = np.ascontiguousarray(np.asarray(inputs["mask_beta"], dtype=np.float32))
    protos = np.ascontiguousarray(
        np.asarray(inputs["prototypes"], dtype=np.float32).reshape(KM, D))

    has_beta = bool(np.any(b != 0.0))
    if has_beta:
        nc = build(has_beta=True)
    else:
        nc = _get_nc()

    blka = np.zeros((KM, K), np.float32)
    blkb = np.zeros((K, KM), np.float32)
    for k in range(K):
        blka[k * M:(k + 1) * M, k] = 1.0
        blkb[k, k * M:(k + 1) * M] = 1.0
    featsT = np.ascontiguousarray(_to_f32r(feats).T)
    xbf = feats.astype(ml_dtypes.bfloat16)
    in_maps = []
    for c in range(NC_CORES):
        sl = slice(c * NLOC, (c + 1) * NLOC)
        gtc = np.ascontiguousarray(gt[sl].reshape(NLOC // 128, 128).T)
        in_maps.append({
            "xt": np.ascontiguousarray(featsT[:, sl]),
            "xbf": np.ascontiguousarray(xbf[sl]),
            "gt": gtc,
            "g": g, "b": b, "mg": mg, "mb": mb,
            "protos": protos, "blka": blka, "blkb": blkb,
        })
    res = run_bass_kernel_spmd(nc, in_maps, core_ids=list(range(NC_CORES)),
                               trace=TRACE, tmpdir=TRACE_DIR)
    global LAST_RES
    LAST_RES = res
    outs = res.results
    oseg = np.concatenate([
        o["out_seg"].reshape(128, NLOC // 128, K).transpose(1, 0, 2).reshape(
            NLOC, K) for o in outs], axis=0)
    newp = outs[0]["new_protos"].reshape(K, M, D)
    return oseg, newp
